# revision 1
# baseline (speedup 1.0000x reference)
"""Trainium2 Bass kernel for BinaryPositionEmbedding.

out[i] = sum over set bits b of x_flat[i] of embedding[b]
       = bits[i, :13] @ embedding[:13]           (bits in {0,1})

Strategy (data-parallel over 8 NeuronCores, 4096 rows each). The output
write is the roofline; the correctness gate (rel err < 2e-2) leaves room
to store fp16 instead of f32, halving HBM store traffic per core from
16 MiB (~47 us at 358 GB/s) to 8 MiB (~23.4 us):
  - Host: scale embedding[b] by the exact power of two 2^-b, round to
    bf16 ([13, 1024] rhs), and send the bit matrix as masked values
    (x & 2^b) in {0, 2^b} — exact in bf16 — as a [13, 4096] bf16 lhsT
    per core (106 KB, same bytes as an int16 x replica). A single K=13
    bf16 matmul then reproduces the product to ~1.6e-3 Frobenius
    relative error (fp16 output rounding included), with no on-device
    bit twiddling at all.
  - Device, per core, per 128-row chunk: 2 matmuls (N=512, K=13) into
    one 2-bank PSUM tile, one [128, 1024] PSUM->SBUF fp16-downcasting
    copy alternating ScalarE/DVE so neither engine caps the 23.4 us
    DMA drain, one contiguous 256 KB store per chunk.
  - bits/emb live in partitioned tiles so a following rep's loads only
    WAR-wait on the early chunks' matmuls, keeping the store stream
    saturated across reps; loads ride a non-store DGE ring; tile pools
    are opened once around the whole program (a pool boundary inserts a
    cross-engine barrier, which would stall the pipeline every rep).
  - Host: gather fp16 shards, upcast to f32.
"""

import numpy as np
import ml_dtypes

import concourse.bass as bass
import concourse.mybir as mybir
import concourse.tile as tile
from concourse import bacc
from concourse.bass_utils import run_bass_kernel_spmd

N_CORES = 8
P = 128
D_MODEL = 1024
N_BITS = 13
K = N_BITS
N_TOTAL = 32768
ROWS = N_TOTAL // N_CORES  # 4096 rows per core


def build_program(
    tc,
    out_ap,
    bits_ap,
    emb_ap,
    rows,
    reps=1,
    unroll=False,
    dma_batch=2,       # chunks per output dma_start (per-store issue cost
                       # ~550ns dominates; 512KB stores halve issue count)
    stage_bufs=8,
    psum_bufs=4,       # [128, 1024] f32 tiles: 2 PSUM banks each
    bits_parts=4,      # split bits load so next-rep loads unblock early
    emb_early_chunks=2,  # chunks served by a separate early-loaded emb tile
    half_chunks=0,     # chunks at the start copied+stored per 512-col half
    load_engine="gpsimd",  # ring for input loads (keep off the store ring)
    store_engine="sync",
    nsplit=2,          # matmul N tiles of 1024/nsplit
    act_pattern="AADAD",  # engine per bulk copy, cycled: A=ScalarE, D=DVE
):
    """Emit the program. out_ap [rows, 1024] fp16; bits_ap [13, rows]
    bf16 masked bit values (0 or 2^b); emb_ap [13, 1024] bf16
    (embedding[b] * 2^-b)."""
    nc = tc.nc
    chunks = rows // P
    out_v = out_ap.rearrange("(m c p) d -> m p c d", c=dma_batch, p=P)
    ldq = getattr(nc, load_engine)
    stq = getattr(nc, store_engine)
    nw = D_MODEL // nsplit  # matmul N width

    with (
        tc.tile_pool(name="const", bufs=1) as cpool,
        tc.tile_pool(name="stage", bufs=stage_bufs) as spool,
        tc.tile_pool(name="psum", bufs=psum_bufs, space="PSUM") as ppool,
    ):
        bits_t = cpool.tile([K, rows], mybir.dt.bfloat16)
        emb_e = cpool.tile([K, D_MODEL], mybir.dt.bfloat16)
        emb_m = cpool.tile([K, D_MODEL], mybir.dt.bfloat16)
        part = rows // bits_parts

        def emit_rep():
            # first bits part + early emb first so chunk 0 starts ASAP
            ldq.dma_start(bits_t[:, :part], bits_ap[:, :part])
            ldq.dma_start(emb_e[:], emb_ap)
            if emb_early_chunks < chunks:
                ldq.dma_start(emb_m[:], emb_ap)
            for q in range(1, bits_parts):
                ldq.dma_start(
                    bits_t[:, q * part : (q + 1) * part],
                    bits_ap[:, q * part : (q + 1) * part],
                )

            copy_idx = 0
            for m in range(chunks // dma_batch):
                half = m < half_chunks
                stg = spool.tile([P, dma_batch, D_MODEL], mybir.dt.float16)
                for c in range(dma_batch):
                    n = m * dma_batch + c
                    lhsT = bits_t[:, n * P : (n + 1) * P]
                    emb_t = emb_e if n < emb_early_chunks else emb_m
                    ps = ppool.tile([P, D_MODEL], mybir.dt.float32)
                    for j in range(nsplit):
                        nsl = slice(j * nw, (j + 1) * nw)
                        nc.tensor.matmul(
                            ps[:, nsl], lhsT, emb_t[:, nsl],
                            start=True, stop=True,
                        )
                    if half:
                        # split the chunk over both copy engines and store
                        # per half: fastest possible pipeline restart
                        for j, csl in ((0, slice(0, 512)), (1, slice(512, 1024))):
                            if j == 0:
                                nc.scalar.copy(stg[:, c, csl], ps[:, csl])
                            else:
                                nc.vector.tensor_copy(stg[:, c, csl], ps[:, csl])
                            stq.dma_start(out_v[m, :, c, csl], stg[:, c, csl])
                    else:
                        if act_pattern[copy_idx % len(act_pattern)] == "A":
                            nc.scalar.copy(stg[:, c], ps[:])
                        else:
                            nc.vector.tensor_copy(stg[:, c], ps[:])
                        copy_idx += 1
                if not half:
                    stq.dma_start(out_v[m], stg[:])

        if reps == 1:
            emit_rep()
        elif unroll is True:
            for _ in range(reps):
                emit_rep()
        elif unroll:  # integer: For_i over groups of `unroll` bodies
            assert reps % unroll == 0
            with tc.For_i(0, reps // unroll, 1):
                for _ in range(unroll):
                    emit_rep()
        else:
            with tc.For_i(0, reps, 1):
                emit_rep()


def _build_nc(rows=ROWS, reps=1, unroll=False, sink=False, **body_kwargs):
    """sink=True (bench-only): the [rows, 1024] result goes to an Internal
    DRAM buffer (same device work, same HBM traffic) and only a [1, 1]
    tick is an ExternalOutput, so per-call host<->device traffic doesn't
    drown the timing signal under the axon tunnel."""
    nc = bacc.Bacc(
        "TRN2", target_bir_lowering=False, debug=False, enable_asserts=False
    )
    bits_in = nc.dram_tensor(
        "bitsbf", [K, rows], mybir.dt.bfloat16, kind="ExternalInput"
    )
    emb_in = nc.dram_tensor(
        "embs", [K, D_MODEL], mybir.dt.bfloat16, kind="ExternalInput"
    )
    out = nc.dram_tensor(
        "out",
        [rows, D_MODEL],
        mybir.dt.float16,
        kind="Internal" if sink else "ExternalOutput",
    )
    tick = (
        nc.dram_tensor("tick", [1, 1], mybir.dt.float16, kind="ExternalOutput")
        if sink
        else None
    )
    with tile.TileContext(nc) as tc:
        build_program(
            tc, out.ap(), bits_in.ap(), emb_in.ap(), rows,
            reps=reps, unroll=unroll, **body_kwargs,
        )
        if sink:
            nc.sync.dma_start(tick.ap(), out.ap()[0:1, 0:1])
    nc.finalize()
    return nc


_NC_CACHE = {}


def make_in_maps(x, embedding):
    x_flat = np.asarray(x).reshape(-1).astype(np.int32)
    emb13 = np.asarray(embedding)[:N_BITS].astype(np.float32)
    # bits arrive as 0 or 2^b (exact in bf16); fold the exact 2^-b scale
    # into the table
    scaled = emb13 * (0.5 ** np.arange(N_BITS, dtype=np.float32))[:, None]
    embs = np.ascontiguousarray(scaled.astype(ml_dtypes.bfloat16))
    masks = (1 << np.arange(K, dtype=np.int32))[:, None]
    bits_all = (x_flat[None, :] & masks).astype(ml_dtypes.bfloat16)  # [13, N]
    in_maps = []
    for c in range(N_CORES):
        in_maps.append(
            {
                "bitsbf": np.ascontiguousarray(
                    bits_all[:, c * ROWS : (c + 1) * ROWS]
                ),
                "embs": embs,
            }
        )
    return in_maps


def kernel(x, embedding, **run_kwargs):
    if "nc" not in _NC_CACHE:
        _NC_CACHE["nc"] = _build_nc()
    nc = _NC_CACHE["nc"]
    in_maps = make_in_maps(x, embedding)
    res = run_bass_kernel_spmd(
        nc, in_maps, core_ids=list(range(N_CORES)), **run_kwargs
    )
    out = np.concatenate(
        [np.asarray(r["out"], dtype=np.float32) for r in res.results], axis=0
    )
    if run_kwargs:
        kernel.last_results = res
    return out



# revision 2
# speedup vs baseline: 1.3367x; 1.3367x over previous
"""Trainium2 Bass kernel for BinaryPositionEmbedding.

out[i] = sum over set bits b of x_flat[i] of embedding[b]
       = bits[i, :13] @ embedding[:13]           (bits in {0,1})

Strategy (data-parallel over 8 NeuronCores, 4096 rows each). The fp16
output store is the roofline (~8 MiB/core at ~360 GB/s ≈ 23.3 us); the
kernel is organized so the store DMA stream never starves:

  - Host: error-compensated fp8 operands. The embedding table is split
    emb ≈ hi + lo/64 with hi = e4m3(emb), lo = e4m3((emb - hi) * 64),
    packed as a [13, 2, 1024] rhs. The bit matrix is packed
    [13, 2, rows] with plane 0 = bit (0/1) and plane 1 = bit * 2^-6
    (both exact in e4m3), so a single DoubleRow matmul per 512-wide
    PSUM half contracts over both planes at 0.5 cycles/column — 2x the
    bf16 column rate — and reproduces bits @ emb to ~7e-4 Frobenius
    relative error (fp16 store rounding included).
  - Device, per core, per 128-row chunk: 2 DoubleRow matmuls (N=512,
    K=13x2) into one 2-bank PSUM tile, one [128, 1024] PSUM->SBUF
    fp16-downcasting copy alternating ScalarE/DVE, one contiguous
    512 KB store per 2-chunk batch on the SP HWDGE ring.
  - bits/emb live in parity-double-buffered SBUF tiles: each rep body
    prefetches the other parity's tiles for the following rep, so at a
    rep boundary the first matmul's inputs are already resident and the
    store stream restarts without draining the load->matmul->copy
    latency chain. Loads ride the Pool SWDGE ring (off the store ring).
    Tile pools are opened once around the whole program.
  - Host: gather fp16 shards, upcast to f32.
"""

import numpy as np
import ml_dtypes

import concourse.bass as bass
import concourse.mybir as mybir
import concourse.tile as tile
from concourse import bacc
from concourse.bass_utils import run_bass_kernel_spmd

N_CORES = 8
P = 128
D_MODEL = 1024
N_BITS = 13
K = N_BITS
N_TOTAL = 32768
ROWS = N_TOTAL // N_CORES  # 4096 rows per core
LO_SCALE = 64.0  # lo plane carries (emb - hi) * 64, bits plane 1 = bit / 64


def build_program(
    tc,
    out_ap,
    bits_ap,
    emb_ap,
    rows,
    reps=1,
    unroll=2,        # reps per For_i iteration; must be even (parity pairs)
    dma_batch=2,     # chunks per output dma_start
    stage_bufs=8,
    psum_bufs=4,     # [128, 1024] f32 tiles: 2 PSUM banks each
    bits_parts=2,    # split bits load into parts
    load_engine="gpsimd",   # ring for input loads (keep off the store ring)
    store_engine="sync",
    act_pattern="AD",  # engine per bulk copy, cycled: A=ScalarE, D=DVE
):
    """Emit the program. out_ap [rows, 1024] fp16; bits_ap [13, 2, rows]
    fp8e4 (plane 0: bit, plane 1: bit/64); emb_ap [13, 2, 1024] fp8e4
    (plane 0: e4m3(emb), plane 1: e4m3((emb - hi) * 64))."""
    nc = tc.nc
    chunks = rows // P
    out_v = out_ap.rearrange("(m c p) d -> m p c d", c=dma_batch, p=P)
    ldq = getattr(nc, load_engine)
    stq = getattr(nc, store_engine)
    f8 = mybir.dt.float8e4
    part = rows // bits_parts

    with (
        tc.tile_pool(name="const", bufs=1) as cpool,
        tc.tile_pool(name="stage", bufs=stage_bufs) as spool,
        tc.tile_pool(name="psum", bufs=psum_bufs, space="PSUM") as ppool,
    ):
        bits_bufs = [
            cpool.tile([K, 2, rows], f8, name=f"bits{i}") for i in range(2)
        ]
        emb_bufs = [
            cpool.tile([K, 2, D_MODEL], f8, name=f"emb{i}") for i in range(2)
        ]

        def load(p):
            ldq.dma_start(emb_bufs[p][:], emb_ap)
            for q in range(bits_parts):
                sl = slice(q * part, (q + 1) * part)
                ldq.dma_start(bits_bufs[p][:, :, sl], bits_ap[:, :, sl])

        def body(p, prefetch=True):
            if prefetch:
                load(1 - p)
            bits_t, emb_t = bits_bufs[p], emb_bufs[p]
            ci = 0
            for m in range(chunks // dma_batch):
                stg = spool.tile(
                    [P, dma_batch, D_MODEL], mybir.dt.float16, name="stg"
                )
                for c in range(dma_batch):
                    n = m * dma_batch + c
                    lhsT = bits_t[:, :, n * P : (n + 1) * P]
                    ps = ppool.tile([P, D_MODEL], mybir.dt.float32, name="ps")
                    for j in range(2):
                        nsl = slice(j * 512, (j + 1) * 512)
                        nc.tensor.matmul(
                            ps[:, nsl],
                            lhsT,
                            emb_t[:, :, nsl],
                            start=True,
                            stop=True,
                            perf_mode=mybir.MatmulPerfMode.DoubleRow,
                        )
                    if act_pattern[ci % len(act_pattern)] == "A":
                        nc.scalar.copy(stg[:, c], ps[:])
                    else:
                        nc.vector.tensor_copy(stg[:, c], ps[:])
                    ci += 1
                stq.dma_start(out_v[m], stg[:])

        load(0)
        if reps == 1:
            body(0, prefetch=False)
        elif unroll is True:
            for r in range(reps):
                body(r % 2)
        else:
            assert unroll % 2 == 0 and reps % unroll == 0, (reps, unroll)
            with tc.For_i(0, reps // unroll, 1):
                for r in range(unroll):
                    body(r % 2)


def _build_nc(rows=ROWS, reps=1, **body_kwargs):
    nc = bacc.Bacc(
        "TRN2", target_bir_lowering=False, debug=False, enable_asserts=False
    )
    bits_in = nc.dram_tensor(
        "bitsf8", [K, 2, rows], mybir.dt.float8e4, kind="ExternalInput"
    )
    emb_in = nc.dram_tensor(
        "embs", [K, 2, D_MODEL], mybir.dt.float8e4, kind="ExternalInput"
    )
    out = nc.dram_tensor(
        "out", [rows, D_MODEL], mybir.dt.float16, kind="ExternalOutput"
    )
    with tile.TileContext(nc) as tc:
        build_program(
            tc, out.ap(), bits_in.ap(), emb_in.ap(), rows,
            reps=reps, **body_kwargs,
        )
    nc.finalize()
    return nc


_NC_CACHE = {}


def make_in_maps(x, embedding):
    f8 = ml_dtypes.float8_e4m3
    x_flat = np.asarray(x).reshape(-1).astype(np.int32)
    emb13 = np.asarray(embedding)[:N_BITS].astype(np.float32)
    hi = emb13.astype(f8)
    lo = ((emb13 - hi.astype(np.float32)) * LO_SCALE).astype(f8)
    embs = np.ascontiguousarray(np.stack([hi, lo], axis=1))  # [13, 2, 1024]
    b = ((x_flat[None, :] >> np.arange(K, dtype=np.int32)[:, None]) & 1).astype(
        np.float32
    )  # [13, N]
    bits_pk = np.stack([b, b * (1.0 / LO_SCALE)], axis=1).astype(f8)
    in_maps = []
    for c in range(N_CORES):
        in_maps.append(
            {
                "bitsf8": np.ascontiguousarray(
                    bits_pk[:, :, c * ROWS : (c + 1) * ROWS]
                ),
                "embs": embs,
            }
        )
    return in_maps


def kernel(x, embedding, **run_kwargs):
    if "nc" not in _NC_CACHE:
        _NC_CACHE["nc"] = _build_nc()
    nc = _NC_CACHE["nc"]
    in_maps = make_in_maps(x, embedding)
    res = run_bass_kernel_spmd(
        nc, in_maps, core_ids=list(range(N_CORES)), **run_kwargs
    )
    out = np.concatenate(
        [np.asarray(r["out"], dtype=np.float32) for r in res.results], axis=0
    )
    if run_kwargs:
        kernel.last_results = res
    return out
